# revision 10
# baseline (speedup 1.0000x reference)
"""Trainium2 Bass kernel for nn_MultiHeadAttention_91285234909775.

Full inputs in, full output out. Sharding: 8 cores = 2 batches x 4 head-groups.
Each core computes QKV projections + attention for its 4 heads of its batch,
then a single 8-core AllToAll reshards from head-parallel to sequence-parallel
for the output projection (each core finishes a 512-row slice of its batch).

Self-contained: hardcodes shapes from the problem spec.
"""

import sys

sys.path.insert(0, "/opt/trn_rl_repo")

import numpy as np
import concourse.bass as bass
import concourse.bacc as bacc
import concourse.tile as tile
import concourse.mybir as mybir
from concourse.bass_utils import run_bass_kernel_spmd

BS, L, DIM, H, POS = 2, 2048, 1024, 16, 16
DH = DIM // H                      # 64
NCORES = 8
HPC = H // 4                       # 4 heads per core
FPC = HPC * DH                     # 256 projected features per core
LQC = L // 4                       # 512-row output slice per core
SCALE = 8.0                        # round(sqrt(64), 2)
NEG = -1.0e9

f32 = mybir.dt.float32
f32r = mybir.dt.float32r
Act = mybir.ActivationFunctionType
Alu = mybir.AluOpType

_cache = {}


def _build(causal: bool):
    nc = bacc.Bacc("TRN2", target_bir_lowering=False, debug=False,
                   num_devices=NCORES)

    # ---- per-core DRAM I/O ----
    qT_d = nc.dram_tensor("qT", [DIM, L], f32r, kind="ExternalInput")
    kT_d = nc.dram_tensor("kT", [DIM, L], f32r, kind="ExternalInput")
    vT_d = nc.dram_tensor("vT", [DIM, L], f32r, kind="ExternalInput")
    enc_d = nc.dram_tensor("encN", [L, POS], f32r, kind="ExternalInput")
    wq_d = nc.dram_tensor("wqT", [DIM, FPC], f32r, kind="ExternalInput")
    wk_d = nc.dram_tensor("wkT", [DIM, FPC], f32r, kind="ExternalInput")
    wv_d = nc.dram_tensor("wvT", [DIM, FPC], f32r, kind="ExternalInput")
    wo_d = nc.dram_tensor("woT", [DIM, DIM], f32r, kind="ExternalInput")
    bq_d = nc.dram_tensor("bq2", [128, 2], f32, kind="ExternalInput")
    bk_d = nc.dram_tensor("bk2", [128, 2], f32, kind="ExternalInput")
    bv_d = nc.dram_tensor("bv1", [1, FPC], f32, kind="ExternalInput")
    bo_d = nc.dram_tensor("bo1", [1, DIM], f32, kind="ExternalInput")
    sel_d = nc.dram_tensor("sel8", [1, 8], f32, kind="ExternalInput")
    id_d = nc.dram_tensor("ident", [128, 128], f32r, kind="ExternalInput")
    if causal:
        mk_d = nc.dram_tensor("masks", [4, 128, 512], f32, kind="ExternalInput")
    else:
        mk_d = nc.dram_tensor("masks", [16, 4, 128, 512], f32,
                              kind="ExternalInput")
    out_d = nc.dram_tensor("out", [LQC, DIM + POS], f32, kind="ExternalOutput")

    def nkt(qc):
        return 4 * qc + 4 if causal else 16

    with tile.TileContext(nc) as tc:
        with (
            tc.tile_pool(name="persist", bufs=1) as pp,
            tc.tile_pool(name="dram", bufs=1, space="DRAM") as dram,
        ):
            # ---- persistent SBUF tensors ----
            wq_sb = pp.tile([128, 8, FPC], f32r)
            wk_sb = pp.tile([128, 8, FPC], f32r)
            wv_sb = pp.tile([128, 8, FPC], f32r)
            wo_sb = pp.tile([128, 8, DIM], f32r)
            qt_sb = pp.tile([128, 2, L], f32r)      # Q^T  [feat, L]
            kt_sb = pp.tile([128, 2, L], f32r)      # K^T  [feat, L]
            vstage = pp.tile([128, 16, HPC, 128], f32r)  # [1|pad|enc|pad|V] per (kt,h)
            bq_sb = pp.tile([128, 2], f32)
            bk_sb = pp.tile([128, 2], f32)
            bv_sb = pp.tile([1, FPC], f32)
            bo_sb = pp.tile([1, DIM], f32)
            bvb = pp.tile([128, FPC], f32)
            bob = pp.tile([128, DIM], f32)
            sel_sb = pp.tile([1, 8], f32)
            selb = pp.tile([128, 8], f32)
            id_sb = pp.tile([128, 128], f32r)
            if causal:
                mk_sb = pp.tile([128, 4, 512], f32)
            else:
                mk_sb = pp.tile([128, 16, 4, 512], f32)
            enc_acc = pp.tile([128, 4, 512], f32r)  # rows 64:80 used

            a2a_in = dram.tile([8, 272, 512], f32r)
            a2a_out = dram.tile([8, 272, 512], f32r)

            # ---- small loads ----
            for ci in range(8):
                nc.sync.dma_start(wq_sb[:, ci, :], wq_d[128 * ci:128 * ci + 128, :])
                nc.sync.dma_start(wk_sb[:, ci, :], wk_d[128 * ci:128 * ci + 128, :])
                nc.sync.dma_start(wv_sb[:, ci, :], wv_d[128 * ci:128 * ci + 128, :])
                nc.sync.dma_start(wo_sb[:, ci, :], wo_d[128 * ci:128 * ci + 128, :])
            nc.sync.dma_start(bq_sb[:], bq_d[:])
            nc.sync.dma_start(bk_sb[:], bk_d[:])
            nc.sync.dma_start(bv_sb[:], bv_d[:])
            nc.sync.dma_start(bo_sb[:], bo_d[:])
            nc.sync.dma_start(sel_sb[:], sel_d[:])
            nc.sync.dma_start(id_sb[:], id_d[:])
            if causal:
                for d in range(4):
                    nc.sync.dma_start(mk_sb[:, d, :], mk_d[d, :, :])
            else:
                for kt in range(16):
                    for qc in range(4):
                        nc.sync.dma_start(mk_sb[:, kt, qc, :], mk_d[kt, qc, :, :])
            nc.gpsimd.partition_broadcast(bvb[:], bv_sb[:])
            nc.gpsimd.partition_broadcast(bob[:], bo_sb[:])
            nc.gpsimd.partition_broadcast(selb[:], sel_sb[:])
            nc.vector.memset(vstage[:, :, :, 1:32].bitcast(f32), 0.0)
            nc.vector.memset(vstage[:, :, :, 48:64].bitcast(f32), 0.0)
            nc.vector.memset(vstage[:, :, :, 0:1].bitcast(f32), 1.0)

            # =====================================================
            # Phase 1: projections
            # =====================================================
            with (
                tc.tile_pool(name="stream", bufs=3) as sp,
                tc.tile_pool(name="vtp", bufs=4) as vtp,
                tc.tile_pool(name="ppsum", bufs=8, space="PSUM") as ppsum,
            ):
                # --- K^T = (Wk^T)^T @ k^T ---
                for name, src_d, w_sb, dst_sb, b_sb in (
                    ("k", kT_d, wk_sb, kt_sb, bk_sb),
                    ("q", qT_d, wq_sb, qt_sb, bq_sb),
                ):
                    psums = [ppsum.tile([128, 512], f32, name=f"ps_{name}{i}", tag="ps")
                             for i in range(8)]
                    for ci in range(8):
                        xt = sp.tile([128, L], f32r, name=f"x_{name}", tag="xs")
                        nc.sync.dma_start(xt[:], src_d[128 * ci:128 * ci + 128, :])
                        for jt in range(2):
                            for qc in range(4):
                                nc.tensor.matmul(
                                    psums[4 * jt + qc][:],
                                    w_sb[:, ci, 128 * jt:128 * jt + 128],
                                    xt[:, 512 * qc:512 * qc + 512],
                                    start=(ci == 0), stop=(ci == 7),
                                )
                    for jt in range(2):
                        for qc in range(4):
                            nc.scalar.activation(
                                dst_sb[:, jt, 512 * qc:512 * qc + 512],
                                psums[4 * jt + qc][:],
                                Act.Identity, bias=b_sb[:, jt:jt + 1],
                            )

                # --- V natural [Lk, feat], built per 128-row tile ---
                vpsums = [ppsum.tile([128, 512], f32, name=f"ps_v{i}", tag="ps")
                          for i in range(8)]
                for kt in range(16):
                    ps = vpsums[kt % 8][:, 256 * (kt // 8):256 * (kt // 8) + 256]
                    for ci in range(8):
                        vt = vtp.tile([128, 128], f32r, name="v_t", tag="vt")
                        nc.sync.dma_start(
                            vt[:], vT_d[128 * ci:128 * ci + 128,
                                        128 * kt:128 * kt + 128])
                        nc.tensor.matmul(
                            ps, vt[:], wv_sb[:, ci, :],
                            start=(ci == 0), stop=(ci == 7),
                        )
                    nc.vector.tensor_add(
                        vstage[:, kt, :, 64:128],
                        ps.rearrange("p (h d) -> p h d", h=HPC),
                        bvb[:].rearrange("p (h d) -> p h d", h=HPC),
                    )
                    for h in range(HPC):
                        nc.sync.dma_start(
                            vstage[:, kt, h, 32:48],
                            enc_d[128 * kt:128 * kt + 128, :])

            # =====================================================
            # Phase 2: attention per (head, q-chunk)
            # =====================================================
            with (
                tc.tile_pool(name="stp", bufs=2, space="PSUM") as stp,
                tc.tile_pool(name="cxp", bufs=2, space="PSUM") as cxp,
                tc.tile_pool(name="ptp", bufs=3) as ptp,
                tc.tile_pool(name="npool", bufs=2) as npool,
                tc.tile_pool(name="cno", bufs=2) as cno,
            ):
                for h in range(HPC):
                    jt, pb = h // 2, 64 * (h % 2)
                    for qc in range(4):
                        n = nkt(qc)
                        ctx = cxp.tile([128, 512], f32, name="ctx")
                        for kt in range(n):
                            st = stp.tile([128, 512], f32, name="st")
                            nc.tensor.matmul(
                                st[:],
                                kt_sb[pb:pb + 64, jt, 128 * kt:128 * kt + 128],
                                qt_sb[pb:pb + 64, jt, 512 * qc:512 * qc + 512],
                                start=True, stop=True,
                            )
                            if causal:
                                if kt >= 4 * qc:
                                    nc.vector.tensor_add(
                                        st[:], st[:], mk_sb[:, kt - 4 * qc, :])
                            else:
                                nc.vector.tensor_add(
                                    st[:], st[:], mk_sb[:, kt, qc, :])
                            pt = ptp.tile([128, 512], f32r, name="pt")
                            nc.scalar.activation(
                                pt[:], st[:], Act.Exp, scale=1.0 / SCALE)
                            nc.tensor.matmul(
                                ctx[0:128, :], vstage[:, kt, h, :], pt[:],
                                start=(kt == 0), stop=(kt == n - 1),
                            )
                        # normalize: row 80 is the softmax denominator
                        rec = npool.tile([128, 512], f32, name="rec")
                        nc.vector.reciprocal(rec[0:1, :], ctx[0:1, :])
                        recb = npool.tile([128, 512], f32, name="recb")
                        nc.gpsimd.partition_broadcast(recb[:], rec[0:1, :])
                        ctxn = cno.tile([128, 512], f32r, name="ctxn")
                        nc.vector.tensor_mul(ctxn[64:128, :], ctx[64:128, :],
                                             recb[64:128, :])
                        if h == 0:
                            nc.vector.tensor_mul(
                                enc_acc[32:48, qc, :], ctx[32:48, :],
                                recb[32:48, :])
                        else:
                            tmp = cno.tile([128, 512], f32, name="etmp")
                            nc.vector.tensor_mul(tmp[32:48, :], ctx[32:48, :],
                                                 recb[32:48, :])
                            nc.vector.tensor_add(
                                enc_acc[32:48, qc, :], enc_acc[32:48, qc, :],
                                tmp[32:48, :])
                        for j in (qc, qc + 4):
                            nc.sync.dma_start(
                                a2a_in[j, 64 * h:64 * h + 64, :], ctxn[64:128, :])
                for qc in range(4):
                    for j in (qc, qc + 4):
                        nc.sync.dma_start(a2a_in[j, 256:272, :],
                                          enc_acc[32:48, qc, :])

            # =====================================================
            # Phase 3: AllToAll (head-parallel -> sequence-parallel)
            # =====================================================
            nc.gpsimd.collective_compute(
                "AllToAll", Alu.bypass,
                replica_groups=[list(range(NCORES))],
                ins=[a2a_in[:].opt()],
                outs=[a2a_out[:].opt()],
            )

            # =====================================================
            # Phase 4: output projection for this core's 512 rows
            # =====================================================
            with (
                tc.tile_pool(name="ldp", bufs=2) as ldp,
                tc.tile_pool(name="asmp", bufs=1) as asmp,
                tc.tile_pool(name="encp", bufs=1) as encp,
                tc.tile_pool(name="outp", bufs=2) as outp,
                tc.tile_pool(name="opsum", bufs=2, space="PSUM") as opsum,
                tc.tile_pool(name="tpsum", bufs=2, space="PSUM") as tpsum,
            ):
                asm = asmp.tile([128, 8, 512], f32r)
                ence = encp.tile([16, 6, 512], f32r)
                for g in range(4):
                    for p in range(2):
                        t0 = ldp.tile([128, 512], f32r, name="t0", tag="ld", bufs=4)
                        t1 = ldp.tile([128, 512], f32r, name="t1", tag="ld", bufs=4)
                        nc.sync.dma_start(
                            t0[:], a2a_out[g, 128 * p:128 * p + 128, :])
                        nc.sync.dma_start(
                            t1[:], a2a_out[g + 4, 128 * p:128 * p + 128, :])
                        tm = ldp.tile([128, 512], f32r, name="tm", tag="ld2")
                        nc.vector.tensor_scalar_mul(
                            tm[:], t1[:], selb[:, g + 4:g + 5])
                        nc.vector.scalar_tensor_tensor(
                            asm[:, 2 * g + p, :], t0[:], selb[:, g:g + 1],
                            tm[:], Alu.mult, Alu.add)
                    e0 = ldp.tile([16, 512], f32r, name="e0", tag="lde")
                    e1 = ldp.tile([16, 512], f32r, name="e1", tag="lde")
                    nc.sync.dma_start(e0[:], a2a_out[g, 256:272, :])
                    nc.sync.dma_start(e1[:], a2a_out[g + 4, 256:272, :])
                    em = ldp.tile([16, 512], f32r, name="em", tag="lde2")
                    nc.vector.tensor_scalar_mul(
                        em[:], e1[:], selb[0:16, g + 4:g + 5])
                    nc.vector.scalar_tensor_tensor(
                        ence[:, g, :], e0[:], selb[0:16, g:g + 1],
                        em[:], Alu.mult, Alu.add)
                nc.vector.tensor_add(ence[:, 4, :], ence[:, 0, :], ence[:, 1, :])
                nc.vector.tensor_add(ence[:, 5, :], ence[:, 2, :], ence[:, 3, :])
                nc.vector.tensor_add(ence[:, 4, :], ence[:, 4, :], ence[:, 5, :])
                nc.vector.tensor_scalar_mul(ence[:, 4, :], ence[:, 4, :],
                                            1.0 / H)

                for lt in range(4):
                    ot = outp.tile([128, DIM + POS], f32, name="ot")
                    for nn in range(2):
                        ops = opsum.tile([128, 512], f32, name="ops")
                        for ci in range(8):
                            nc.tensor.matmul(
                                ops[:],
                                asm[:, ci, 128 * lt:128 * lt + 128],
                                wo_sb[:, ci, 512 * nn:512 * nn + 512],
                                start=(ci == 0), stop=(ci == 7),
                            )
                        nc.vector.tensor_add(
                            ot[:, 512 * nn:512 * nn + 512], ops[:],
                            bob[:, 512 * nn:512 * nn + 512])
                    tp = tpsum.tile([128, 16], f32r, name="tp")
                    nc.tensor.transpose(
                        tp[:], ence[:, 4, 128 * lt:128 * lt + 128],
                        id_sb[0:16, 0:16])
                    nc.vector.tensor_copy(ot[:, DIM:DIM + POS], tp[:])
                    nc.sync.dma_start(out_d[128 * lt:128 * lt + 128, :], ot[:])

    nc.compile()
    return nc


def _get_nc(causal: bool):
    if causal not in _cache:
        _cache[causal] = _build(causal)
    return _cache[causal]


def _causal_masks():
    # diagonal-block additive masks: tile d, entry (k, q):
    # valid iff 128*d + k <= q
    d = np.arange(4)[:, None, None]
    k = np.arange(128)[None, :, None]
    q = np.arange(512)[None, None, :]
    return np.where(128 * d + k <= q, 0.0, NEG).astype(np.float32)


def _dense_masks(attn_mask):
    # mask[q, k] == 1 -> NEG ; tiles indexed [kt, qc, k(128), q(512)]
    madd = np.where(attn_mask.T == 1, NEG, 0.0).astype(np.float32)  # [k, q]
    return np.ascontiguousarray(
        madd.reshape(16, 128, 4, 512).transpose(0, 2, 1, 3))


def kernel(**inputs):
    q = np.asarray(inputs["q"], np.float32)
    k = np.asarray(inputs["k"], np.float32)
    v = np.asarray(inputs["v"], np.float32)
    Wq = np.asarray(inputs["Wq"], np.float32)
    bq = np.asarray(inputs["bq"], np.float32)
    Wk = np.asarray(inputs["Wk"], np.float32)
    bk = np.asarray(inputs["bk"], np.float32)
    Wv = np.asarray(inputs["Wv"], np.float32)
    bv = np.asarray(inputs["bv"], np.float32)
    Wo = np.asarray(inputs["Wo"], np.float32)
    bo = np.asarray(inputs["bo"], np.float32)
    attn_mask = np.asarray(inputs["attn_mask"])
    pos = int(inputs["pos_token_dim"])
    assert pos == POS and q.shape == (BS, L, DIM + POS)

    causal = bool(
        np.array_equal(attn_mask,
                       np.triu(np.ones((L, L), attn_mask.dtype), k=1)))
    nc = _get_nc(causal)

    masks = _causal_masks() if causal else _dense_masks(attn_mask)
    ident = np.eye(128, dtype=np.float32)

    in_maps = []
    for c in range(NCORES):
        b, hg = c // 4, c % 4
        fsl = slice(FPC * hg, FPC * hg + FPC)
        in_maps.append({
            "qT": np.ascontiguousarray(q[b, :, :DIM].T),
            "kT": np.ascontiguousarray(k[b, :, :DIM].T),
            "vT": np.ascontiguousarray(v[b, :, :DIM].T),
            "encN": np.ascontiguousarray(k[b, :, DIM:]),
            "wqT": np.ascontiguousarray(Wq.T[:, fsl]),
            "wkT": np.ascontiguousarray(Wk.T[:, fsl]),
            "wvT": np.ascontiguousarray(Wv.T[:, fsl]),
            "woT": np.ascontiguousarray(Wo.T),
            "bq2": np.ascontiguousarray(bq[fsl].reshape(2, 128).T),
            "bk2": np.ascontiguousarray(bk[fsl].reshape(2, 128).T),
            "bv1": np.ascontiguousarray(bv[fsl][None, :]),
            "bo1": np.ascontiguousarray(bo[None, :]),
            "sel8": np.array([[1.0] * 4 + [0.0] * 4] if b == 0
                             else [[0.0] * 4 + [1.0] * 4], np.float32),
            "ident": ident,
            "masks": masks,
        })

    global _last_in_maps
    _last_in_maps = in_maps
    res = run_bass_kernel_spmd(nc, in_maps, core_ids=list(range(NCORES)))
    out = np.empty((BS, L, DIM + POS), np.float32)
    for c in range(NCORES):
        b, hg = c // 4, c % 4
        out[b, LQC * hg:LQC * hg + LQC, :] = res.results[c]["out"]
    return out
